# revision 51
# baseline (speedup 1.0000x reference)
"""AFT-Local sparse attention kernel for Trainium2, SPMD over 8 NeuronCores.

Problem (B=4, L=1024, E=256, S=32):
    Q = q @ Wq.T + bq ; K = q @ Wk.T + bk ; V = q @ Wv.T + bv
    For each (b, i, e):  per-channel softmax over the 65-wide window
        logits[j] = Q[i,e] * (K[i+j-S, e] + pb[j, e])   for |j-S| < S (strict)
        logits[j] = 0                                    for j in {0, 64} (K masked)
        logits[j] = -inf                                 for out-of-range positions
        ctx = sum_j softmax(logits)[j] * V[i+j-S, e]
    out = sigmoid(Q)^2 * ctx

Sharding: 8 cores = (batch b in 0..3) x (sequence half h in 0..1).
The h=1 half is REVERSED on the host so that every core sees an identical
problem: a sequence edge at local position 0 and valid data through the
right halo.  This keeps the SPMD graph uniform (no per-core masking).

Device layout: channels on partitions (2 halves of 128), sequence on the
free axis.  Window shifts are free AP offsets.  Per window offset d:
  DVE/ACT:  t_d = K<<d + pb[d]             (tensor_scalar slabs, split
                                            between the two engines)
  DVE:  l_d = t_d * Q                      (one fused G-slab tensor_tensor)
  ACT:  E_d = exp(l_d)
  DVE:  EV_d = E_d * V<<d
  PE:   D += I.T @ E_d ; N += I.T @ EV_d   (identity matmuls accumulate in
        PSUM; D-matmuls first so ln(D) overlaps the N accumulation)
Final: out = sigmoid(Q)^2 * N / D.

The hot path runs in bf16 (measured end-to-end error ~1e-2 vs the 2e-2
gate).  K also exists as a 1-element-shifted copy so reads at odd window
offsets stay 4-byte aligned (keeps the DVE 2x packed mode).

Raw Bass (manual semaphores): this walrus build rejects Tile's generated
sync (multi-wait instructions), so engine programs and cumulative
wait_ge thresholds are written out explicitly.
"""

import contextlib

import ml_dtypes
import numpy as np

import concourse.bass as bass
import concourse.mybir as mybir
from concourse import bass_utils

# Re-enable walrus's LDWEIGHTS elision: the window-sum accumulation issues
# 250+ matmuls with the same identity lhsT, and the default
# --enable-ldw-opt=false reloads it every time.
if not getattr(bass_utils, "_ldw_opt_patched", False):
    bass_utils._ldw_opt_patched = True
    _orig_run_command = bass_utils.run_command

    def _run_command_ldw(argv, **kwargs):
        argv = ["--enable-ldw-opt=true" if a == "--enable-ldw-opt=false"
                else a for a in argv]
        return _orig_run_command(argv, **kwargs)

    bass_utils.run_command = _run_command_ldw

B, L, E, S = 4, 1024, 256, 32
O = 512          # output positions per core
HALO = 32        # halo on each side of the output range
NH = O + 2 * HALO  # 576: local K/V/q array length
P = 128
W = 2 * S + 1
F32 = mybir.dt.float32
BF16 = mybir.dt.bfloat16
NPBF = ml_dtypes.bfloat16

G = 7       # window offsets processed per block
NBLK = 18   # 9 blocks per channel half

TRACE = False
LAST_RESULTS = None
_DEBUG_TAP = None
_CACHE = {}

# ---- static semaphore bookkeeping ----
# sem_pe counts: 24 proj matmuls, then per eh: 3 init + 14 per block
PE_PROJ = 24


def n_act_slabs(b):
    """K+pb slabs done by the ACT engine for block b (rest on the DVE)."""
    if b < 2:
        return 0
    return 2 if b % 2 == 0 else 3


def blk_iters(b):
    return b // 9, -S + 1 + G * (b % 9)


def pe_after_block(b):
    return PE_PROJ + (3 if b < 9 else 6) + 14 * (b + 1)


def pe_d_close(b):
    """PE count at which block b's D-side matmuls have retired (D first)."""
    return pe_after_block(b) - G


PE_TOTAL = PE_PROJ + 6 + 14 * NBLK

# Projection copy-outs: groups 2-3 (eh0 k) on the DVE (s_pd, 1/group, so
# the K1 shift DMA can launch as early as possible); groups 0-1 and 4-11
# on the ScalarE (s_prj; v-chunk0 groups emit 2 ops).
ACT_CUM = {0: 1, 1: 2, 4: 4, 5: 5, 6: 6, 7: 7, 8: 8, 9: 9, 10: 11, 11: 12}


def _build_nc():
    nc = bass.Bass("TRN2")

    ACOLS = 2 * (NH + E)        # 1664: qT0 | Wq0 | qT1 | Wq1 per row
    BCOLS = 4 * E               # 1024: Wk0 | Wv0 | Wk1 | Wv1 per row
    FCOLS = W + 3               # 68:   pbT | biases
    blobA_d = nc.dram_tensor("blobA", [P, ACOLS], BF16, kind="ExternalInput")
    blobB_d = nc.dram_tensor("blobB", [P, BCOLS], BF16, kind="ExternalInput")
    # per-partition scalar operands must be f32 (ISA requirement)
    pbblob_d = nc.dram_tensor("pbblob", [P, 2 * W], F32, kind="ExternalInput")
    bblob_d = nc.dram_tensor("bblob", [P, 6], F32, kind="ExternalInput")
    out_dt = F32 if _DEBUG_TAP is not None else BF16
    out_d = nc.dram_tensor("out", [E, O], out_dt, kind="ExternalOutput")

    add = mybir.AluOpType.add
    mult = mybir.AluOpType.mult
    AF = mybir.ActivationFunctionType

    ctx = contextlib.ExitStack()
    with ctx:
        sb = lambda name, shape, dt=BF16: ctx.enter_context(
            nc.sbuf_tensor(name, shape, dt))[:, :]
        ps = lambda name, shape: ctx.enter_context(
            nc.psum_tensor(name, shape, F32))[:, :]
        sem = lambda name: ctx.enter_context(nc.semaphore(name))

        blobA = sb("blobA_s", [P, 2 * (NH + E)])
        blobB = sb("blobB_s", [P, 4 * E])
        pbs = sb("pbs", [P, 2 * W], F32)
        bss = sb("bss", [P, 6], F32)
        qT = [blobA[:, kh * (NH + E):kh * (NH + E) + NH] for kh in range(2)]
        wT = {("q", kh): blobA[:, kh * (NH + E) + NH:(kh + 1) * (NH + E)]
              for kh in range(2)}
        for kh in range(2):
            wT["k", kh] = blobB[:, 2 * kh * E:(2 * kh + 1) * E]
            wT["v", kh] = blobB[:, (2 * kh + 1) * E:(2 * kh + 2) * E]
        pb = [pbs[:, eh * W:(eh + 1) * W] for eh in range(2)]
        bs = [bss[:, eh * 3:(eh + 1) * 3] for eh in range(2)]
        QKV = {(t, eh): sb(f"{t}{eh}", [P, NH])
               for t in "qkv" for eh in range(2)}
        # 1-element-shifted copy for odd window offsets (alignment)
        K1 = [sb(f"k1_{eh}", [P, NH]) for eh in range(2)]
        ident = sb("ident", [P, P])
        ones2 = sb("ones2", [P, O])
        tbig = [sb(f"tbig{i}", [P, G * O]) for i in range(2)]
        lbig = [sb(f"lbig{i}", [P, G * O]) for i in range(4)]
        ebig = [sb(f"ebig{i}", [P, G * O]) for i in range(4)]
        vbig = [sb(f"vbig{i}", [P, G * O]) for i in range(4)]
        epT = [sb(f"epT{eh}", [P, O], F32) for eh in range(2)]
        epLD = [sb(f"epLD{eh}", [P, O], F32) for eh in range(2)]
        epLU = [sb(f"epLU{eh}", [P, O], F32) for eh in range(2)]
        epS = [sb(f"epS{eh}", [P, O], F32) for eh in range(2)]
        ob = [sb(f"ob{eh}", [P, O], out_dt) for eh in range(2)]
        tapb = sb("tapb", [P, O], F32)

        prj_ps = [ps(f"prj_ps{i}", [P, O]) for i in range(2)]
        D_ps = [ps(f"D_ps{eh}", [P, O]) for eh in range(2)]
        N_ps = [ps(f"N_ps{eh}", [P, O]) for eh in range(2)]

        s_pk = sem("s_pk")  # K1 shifted copies (sync-engine DMA)
        s_lA = sem("s_lA")  # blobA: qT + Wq (2 DMAs, 16 each)
        s_lC = sem("s_lC")  # blobB: Wk + Wv
        s_lB = sem("s_lB")  # bias + pos_bias
        s_gp = sem("s_gp")
        s_prj = sem("s_prj")
        s_pd = sem("s_pd")
        s_lg = sem("s_lg")
        s_ex = sem("s_ex")
        s_ev = sem("s_ev")
        s_pe = sem("s_pe")
        s_ea = sem("s_ea")
        s_ed = sem("s_ed")
        s_epi = sem("s_epi")
        s_od = sem("s_od")
        s_at = sem("s_at")

        def k_sh(eh, d):
            """K window-shifted AP, 4B-aligned: even offsets from K, odd
            from the 1-shifted copy."""
            o = HALO + d
            if o % 2 == 0:
                return QKV["k", eh][:, o:o + O]
            return K1[eh][:, o - 1:o - 1 + O]

        # projection groups: (eh, t, (n0, nn))
        groups = [(eh, t, c) for eh in range(2) for t in "qkv"
                  for c in ((0, 512), (512, NH - 512))]

        with nc.Block() as block:

            @block.sync
            def _(sync):
                # Consolidated loads: per-partition rows are contiguous
                # in DRAM so each DMA moves large packets. blobA is the
                # critical one (projections): split in two so both halves
                # transfer in parallel.
                HC = NH + E
                sync.dma_start(out=blobA[:, 0:HC], in_=blobA_d[:, 0:HC]
                               ).then_inc(s_lA, 16)
                sync.dma_start(out=blobA[:, HC:2 * HC],
                               in_=blobA_d[:, HC:2 * HC]).then_inc(s_lA, 16)
                # small scalar blobs before blobB: blobB is only needed by
                # the PE from projection group 2 on (~2us later).
                sync.dma_start(out=pbs, in_=pbblob_d[:, :]).then_inc(s_lB, 16)
                sync.dma_start(out=bss, in_=bblob_d[:, :]).then_inc(s_lB, 16)
                sync.dma_start(out=blobB, in_=blobB_d[:, :]
                               ).then_inc(s_lC, 16)
                # K1 shifted copies as SBUF->SBUF DMAs: frees the DVE.
                sync.wait_ge(s_pd, 2)
                sync.dma_start(out=K1[0][:, 0:NH - 1],
                               in_=QKV["k", 0][:, 1:NH]).then_inc(s_pk, 16)
                sync.wait_ge(s_prj, 9)
                sync.dma_start(out=K1[1][:, 0:NH - 1],
                               in_=QKV["k", 1][:, 1:NH]).then_inc(s_pk, 16)
                if _DEBUG_TAP is None:
                    H = O // 2
                    sync.wait_ge(s_epi, 1)
                    sync.dma_start(out=out_d[0:P, :], in_=ob[0]
                                   ).then_inc(s_od, 16)
                    # eh1 leaves in two halves so the first DMA overlaps
                    # the second half's multiply.
                    sync.wait_ge(s_epi, 2)
                    sync.dma_start(out=out_d[P:2 * P, 0:H], in_=ob[1][:, 0:H]
                                   ).then_inc(s_od, 16)
                    sync.wait_ge(s_epi, 3)
                    sync.dma_start(out=out_d[P:2 * P, H:O], in_=ob[1][:, H:O]
                                   ).then_inc(s_od, 16)
                    sync.wait_ge(s_od, 48)
                else:
                    sync.wait_ge(s_epi, 3)
                    tap = {
                        "D0": lambda: tapb,
                        "N0": lambda: tapb,
                        "out0": lambda: ob[0],
                    }[_DEBUG_TAP]()
                    tw = tap.shape[1]
                    sync.dma_start(out=out_d[0:P, 0:tw], in_=tap
                                   ).then_inc(s_od, 16)
                    sync.wait_ge(s_od, 16)

            @block.gpsimd
            def _(gpsimd):
                gpsimd.memset(ident, 0.0)
                gpsimd.affine_select(
                    out=ident, in_=ident,
                    compare_op=mybir.AluOpType.not_equal,
                    fill=1.0, base=0, pattern=[[-1, P]], channel_multiplier=1,
                ).then_inc(s_gp, 1)
                # D-init vector: 1 in the left-edge region (r=-S invalid),
                # 2 elsewhere (both window-edge exp(0) terms).
                gpsimd.memset(ones2, 2.0)
                gpsimd.memset(ones2[:, 0:HALO], 1.0).then_inc(s_gp, 1)

            @block.tensor
            def _(tensor):
                # projections: 4-deep ping-pong (the D/N eh0 banks are free
                # until the window phase starts, which waits s_prj>=10)
                tensor.wait_ge(s_lA, 32)
                kv_waited = False
                pbanks = [prj_ps[0], prj_ps[1], D_ps[0], N_ps[0]]
                for g, (eh, t, (n0, nn)) in enumerate(groups):
                    bank = pbanks[g % 4]
                    if t != "q" and not kv_waited:
                        kv_waited = True
                        tensor.wait_ge(s_lC, 16)
                    if g >= 4:
                        dep = g - 4
                        if dep in (2, 3):
                            tensor.wait_ge(s_pd, dep - 1)
                        else:
                            tensor.wait_ge(s_prj, ACT_CUM[dep])
                    for kh in range(2):
                        tensor.matmul(
                            bank[:, :nn],
                            wT[t, kh][:, eh * P:(eh + 1) * P],
                            qT[kh][:, n0:n0 + nn],
                            start=(kh == 0), stop=(kh == 1),
                        ).then_inc(s_pe, 1)
                # window accumulation, one block of G offsets at a time.
                # Per block: D-side matmuls first (gated on exp only), so
                # the D accumulation closes G matmuls before N and the
                # ln(D) epilogue overlaps the N accumulation.
                for b in range(NBLK):
                    eh, d0 = blk_iters(b)
                    if b % 9 == 0:
                        V = QKV["v", eh]
                        if b == 0:
                            tensor.wait_ge(s_gp, 2)
                        # s_prj>=12 also guarantees the proj bank aliasing
                        # of D_ps[0]/N_ps[0] is fully drained.
                        tensor.wait_ge(s_prj, 12)
                        # r=+S edge term (always valid) zeroes the banks,
                        # then the r=-S edge term and the 1/2 D-init.
                        tensor.matmul(N_ps[eh], ident,
                                      V[:, 2 * HALO:2 * HALO + O],
                                      start=True, stop=False).then_inc(s_pe, 1)
                        tensor.matmul(N_ps[eh][:, HALO:], ident,
                                      V[:, HALO:O],
                                      start=False, stop=False).then_inc(s_pe, 1)
                        tensor.matmul(D_ps[eh], ident, ones2,
                                      start=True, stop=False).then_inc(s_pe, 1)
                    tensor.wait_ge(s_ex, b + 1)
                    for g in range(G):
                        d = d0 + g
                        vs = max(0, -d)
                        tensor.matmul(D_ps[eh][:, vs:], ident,
                                      ebig[b % 4][:, g * O + vs:(g + 1) * O],
                                      start=False, stop=(d == S - 1)
                                      ).then_inc(s_pe, 1)
                    tensor.wait_ge(s_ev, b + 1)
                    for g in range(G):
                        if b == NBLK - 1 and g == 4:
                            tensor.wait_ge(s_ev, b + 2)  # split last EV
                        d = d0 + g
                        vs = max(0, -d)
                        tensor.matmul(N_ps[eh][:, vs:], ident,
                                      vbig[b % 4][:, g * O + vs:(g + 1) * O],
                                      start=False, stop=(d == S - 1)
                                      ).then_inc(s_pe, 1)

            @block.vector
            def _(vector):
                def emit_ev_block(bb, g0=0, gn=G):
                    ehb, d0b = blk_iters(bb)
                    if g0 == 0:
                        vector.wait_ge(s_ex, bb + 1)
                        if bb >= 3 and bb % 2 == 1:
                            # vbig slot free; 4-buffered, so one wait at odd
                            # bb covers this block (needs bb-4) and the next
                            # (needs bb-3).
                            vector.wait_ge(s_pe, pe_after_block(bb - 3))
                    # single op: misaligned packed reads measured full-speed
                    ng = gn - g0
                    vsrc = QKV["v", ehb]
                    in1 = bass.AP(
                        tensor=vsrc.tensor,
                        offset=vsrc.offset + HALO + d0b + g0,
                        ap=[vsrc.ap[0], [1, ng], [1, O]])
                    e3 = bass.AP(
                        tensor=ebig[bb % 4].tensor,
                        offset=ebig[bb % 4].offset + g0 * O,
                        ap=[ebig[bb % 4].ap[0], [O, ng], [1, O]])
                    v3 = bass.AP(
                        tensor=vbig[bb % 4].tensor,
                        offset=vbig[bb % 4].offset + g0 * O,
                        ap=[vbig[bb % 4].ap[0], [O, ng], [1, O]])
                    vector.tensor_tensor(out=v3, in0=e3, in1=in1, op=mult
                                         ).then_inc(s_ev, 1)

                vector.wait_ge(s_lB, 32)  # bias scalars present
                pbanks = [prj_ps[0], prj_ps[1], D_ps[0], N_ps[0]]
                for g in (2, 3):
                    ehg, tg, (n0, nn) = groups[g]
                    ti = "qkv".index(tg)
                    vector.wait_ge(s_pe, 2 * (g + 1))
                    vector.tensor_scalar_add(
                        QKV[tg, ehg][:, n0:n0 + nn], pbanks[g % 4][:, :nn],
                        bs[ehg][:, ti:ti + 1]).then_inc(s_pd, 1)
                for b in range(NBLK):
                    eh, d0 = blk_iters(b)
                    Q = QKV["q", eh]
                    gs = list(range(G - n_act_slabs(b)))
                    if b % 9 == 0:
                        if b == 9:
                            vector.wait_ge(s_prj, 9)  # K+Q eh1 from the ACT
                        # do the direct-K slabs while the K1 DMA lands
                        gs = ([g for g in gs if (HALO + d0 + g) % 2 == 0]
                              + [None]
                              + [g for g in gs if (HALO + d0 + g) % 2 == 1])
                    for g in gs:
                        if g is None:
                            vector.wait_ge(s_pk, 16 * (eh + 1))  # K1 DMA
                            continue
                        d = d0 + g
                        vector.tensor_scalar_add(
                            tbig[b % 2][:, g * O:(g + 1) * O], k_sh(eh, d),
                            pb[eh][:, d + S:d + S + 1])
                    if b >= 2:
                        vector.wait_ge(s_at, b - 1)  # ACT-side K+pb slabs
                    if b >= 4 and b % 2 == 0:
                        # lbig slot free; one wait at even b covers b
                        # (needs s_ex >= b-3) and b+1 (needs b-2).
                        vector.wait_ge(s_ex, b - 2)
                    qb = bass.AP(
                        tensor=Q.tensor, offset=Q.offset + HALO,
                        ap=[Q.ap[0], [0, G], [1, O]])
                    tb3 = bass.AP(
                        tensor=tbig[b % 2].tensor, offset=tbig[b % 2].offset,
                        ap=[tbig[b % 2].ap[0], [O, G], [1, O]])
                    lb3 = bass.AP(
                        tensor=lbig[b % 4].tensor, offset=lbig[b % 4].offset,
                        ap=[lbig[b % 4].ap[0], [O, G], [1, O]])
                    vector.tensor_tensor(out=lb3, in0=tb3, in1=qb, op=mult
                                         ).then_inc(s_lg, 1)
                    if b == 2:
                        vector.wait_ge(s_prj, 5)   # V eh0 for the EV mult
                    if b == 11:
                        vector.wait_ge(s_prj, 12)  # V eh1 for the EV mult
                    if b >= 2:
                        emit_ev_block(b - 2)
                    if b == 2:
                        for eh in range(2):
                            vector.wait_ge(s_ea, eh + 1)
                            vector.tensor_scalar_add(epS[eh], epT[eh], 1.0
                                                     ).then_inc(s_ed, 1)
                    if b == 13:
                        vector.wait_ge(s_ea, 5)
                        vector.scalar_tensor_tensor(
                            out=epT[0], in0=epLU[0], scalar=2.0,
                            in1=epLD[0], op0=mult, op1=add,
                        ).then_inc(s_ed, 1)
                    if b == 15:
                        vector.wait_ge(s_ea, 6)
                        vector.wait_ge(s_pe, pe_after_block(8))  # N0 done
                        vector.tensor_mul(ob[0], N_ps[0], epS[0]
                                          ).then_inc(s_epi, 1)
                emit_ev_block(NBLK - 2)
                # last block's EV in two halves so the PE's final N matmuls
                # start one half earlier
                emit_ev_block(NBLK - 1, 0, 4)
                emit_ev_block(NBLK - 1, 4, G)

                # tail: out = N * exp(-(ln D + 2 ln(1+exp(-Q))))
                if _DEBUG_TAP in ("D0", "N0"):
                    vector.wait_ge(s_pe, PE_TOTAL)
                    vector.tensor_copy(
                        tapb, D_ps[0] if _DEBUG_TAP == "D0" else N_ps[0])
                H = O // 2
                vector.wait_ge(s_ea, 7)
                vector.scalar_tensor_tensor(
                    out=epT[1], in0=epLU[1], scalar=2.0,
                    in1=epLD[1], op0=mult, op1=add,
                ).then_inc(s_ed, 1)
                vector.wait_ge(s_ea, 8)
                vector.wait_ge(s_pe, PE_TOTAL)  # N1 done
                vector.tensor_mul(ob[1][:, 0:H], N_ps[1][:, 0:H],
                                  epS[1][:, 0:H]).then_inc(s_epi, 1)
                vector.wait_ge(s_ea, 9)
                vector.tensor_mul(ob[1][:, H:O], N_ps[1][:, H:O],
                                  epS[1][:, H:O]).then_inc(s_epi, 1)

            @block.scalar
            def _(scalar):
                # projections: add bias, move PSUM -> SBUF (groups 4-11)
                scalar.wait_ge(s_lB, 32)  # bias + pos_bias present
                pbanks = [prj_ps[0], prj_ps[1], D_ps[0], N_ps[0]]
                for g in (0, 1, *range(4, 12)):
                    eh, t, (n0, nn) = groups[g]
                    ti = "qkv".index(t)
                    bank = pbanks[g % 4]
                    scalar.wait_ge(s_pe, 2 * (g + 1))
                    T_sb = QKV[t, eh]
                    if t == "v" and n0 == 0:
                        scalar.activation(T_sb[:, 0:HALO], bank[:, 0:HALO],
                                          AF.Copy).then_inc(s_prj, 1)
                        scalar.activation(
                            T_sb[:, HALO:nn], bank[:, HALO:nn], AF.Identity,
                            bias=bs[eh][:, ti:ti + 1], scale=1.0,
                        ).then_inc(s_prj, 1)
                    else:
                        scalar.activation(
                            T_sb[:, n0:n0 + nn], bank[:, :nn], AF.Identity,
                            bias=bs[eh][:, ti:ti + 1], scale=1.0,
                        ).then_inc(s_prj, 1)
                    if g == 9:
                        # exp(0) as soon as the first logits land; the
                        # remaining copies and sigma-side ops are off the
                        # critical path.
                        scalar.wait_ge(s_lg, 1)
                        scalar.activation(ebig[0], lbig[0], AF.Exp
                                          ).then_inc(s_ex, 1)
                for eh in range(2):
                    scalar.activation(epT[eh], QKV["q", eh][:, HALO:HALO + O],
                                      AF.Exp, scale=-1.0).then_inc(s_ea, 1)
                for b in range(NBLK):
                    ehb, d0b = blk_iters(b)
                    if b >= 2:
                        if b == 2:
                            scalar.wait_ge(s_pk, 16)  # eh0 K1 via DMA
                        if b == 9:
                            scalar.wait_ge(s_pk, 32)  # eh1 K1 via DMA
                        scalar.wait_ge(s_lg, b - 1)  # tbig slot free
                        for g in range(G - n_act_slabs(b), G):
                            d = d0b + g
                            ai = scalar.activation(
                                tbig[b % 2][:, g * O:(g + 1) * O],
                                k_sh(ehb, d), AF.Identity,
                                bias=pb[ehb][:, d + S:d + S + 1], scale=1.0)
                        ai.then_inc(s_at, 1)
                    if b >= 2:  # exp(0) was hoisted above the sigma ops
                        bb = b - 1
                        scalar.wait_ge(s_lg, bb + 1)
                        if bb >= 4 and bb % 2 == 0:
                            # ebig slot free (covers PE reads and the DVE EV
                            # of bb-4, transitively); one wait at even bb
                            # covers bb (needs bb-4) and bb+1 (needs bb-3).
                            scalar.wait_ge(s_pe, pe_after_block(bb - 3))
                        scalar.activation(ebig[bb % 4], lbig[bb % 4], AF.Exp
                                          ).then_inc(s_ex, 1)
                        if bb == 4:
                            # LU = ln(1+exp(-Q)), both eh (u ready early)
                            for eh in range(2):
                                scalar.wait_ge(s_ed, eh + 1)
                                scalar.activation(epLU[eh], epS[eh], AF.Ln
                                                  ).then_inc(s_ea, 1)
                        if bb == 10:
                            scalar.wait_ge(s_pe, pe_d_close(8))
                            scalar.activation(epLD[0], D_ps[0], AF.Ln
                                              ).then_inc(s_ea, 1)
                        if bb == 12:
                            scalar.wait_ge(s_ed, 3)
                            scalar.activation(epS[0], epT[0], AF.Exp,
                                              scale=-1.0).then_inc(s_ea, 1)
                bb = NBLK - 1
                scalar.wait_ge(s_lg, bb + 1)
                # ebig slot covered by the parity wait at bb-1
                scalar.activation(ebig[bb % 4], lbig[bb % 4], AF.Exp
                                  ).then_inc(s_ex, 1)
                # epilogue, same exp/ln table set (no set switch).
                # sigma-side (T = exp(-Q), LU = ln(1+T)) runs early; the
                # D-dependent part starts as soon as block 17's D-side
                # matmuls retire, overlapping the N accumulation.
                scalar.wait_ge(s_pe, pe_d_close(NBLK - 1))
                scalar.activation(epLD[1], D_ps[1], AF.Ln).then_inc(s_ea, 1)
                Hh = O // 2
                scalar.wait_ge(s_ed, 4)
                scalar.activation(epS[1][:, 0:Hh], epT[1][:, 0:Hh], AF.Exp,
                                  scale=-1.0).then_inc(s_ea, 1)
                scalar.activation(epS[1][:, Hh:O], epT[1][:, Hh:O], AF.Exp,
                                  scale=-1.0).then_inc(s_ea, 1)

    return nc


def _shard_inputs(q, Wq, bq, Wk, bk, Wv, bv, pos_bias):
    """Build per-core input maps. Core c = 2*b + h."""
    wqT = Wq.T.astype(NPBF)
    wkT = Wk.T.astype(NPBF)
    wvT = Wv.T.astype(NPBF)
    bias = np.stack([bq, bk, bv], axis=1).astype(np.float32)   # [E, 3]
    bblob = np.ascontiguousarray(
        np.concatenate([bias[0:P], bias[P:E]], axis=1))        # [P, 6]
    pbT_f = pos_bias.T.astype(np.float32)                      # [E, W]
    pbT_r = pos_bias[::-1].T.astype(np.float32)                # reversed

    blobB = np.ascontiguousarray(np.concatenate(
        [wkT[0:P], wvT[0:P], wkT[P:E], wvT[P:E]], axis=1))     # [P, 4E]
    in_maps = []
    for c in range(8):
        b, h = divmod(c, 2)
        qh = np.zeros((NH, E), np.float32)
        if h == 0:
            qh[HALO:] = q[b, 0:O + HALO]          # positions -32..543, pad<0
        else:
            qh[HALO:] = q[b, L - (O + HALO):][::-1]  # reversed right half
        qT = qh.T.astype(NPBF)                                 # [E, NH]
        pbT = pbT_f if h == 0 else pbT_r
        in_maps.append({
            "blobA": np.ascontiguousarray(np.concatenate(
                [qT[0:P], wqT[0:P], qT[P:E], wqT[P:E]], axis=1)),
            "blobB": blobB,
            "pbblob": np.ascontiguousarray(np.concatenate(
                [pbT[0:P], pbT[P:E]], axis=1)),                # [P, 2W]
            "bblob": bblob,
        })
    return in_maps


def _unshard(results):
    out = np.empty((B, L, E), np.float32)
    for c in range(8):
        b, h = divmod(c, 2)
        o_core = np.asarray(results[c]["out"]).astype(np.float32).T  # [O, E]
        if h == 0:
            out[b, 0:O] = o_core
        else:
            out[b, L - O:] = o_core[::-1]
    return out


def kernel(q, Wq, bq, Wk, bk, Wv, bv, pos_bias):
    global LAST_RESULTS
    q = np.asarray(q, np.float32)
    if "nc" not in _CACHE:
        _CACHE["nc"] = _build_nc()
    nc = _CACHE["nc"]
    in_maps = _shard_inputs(q, np.asarray(Wq), np.asarray(bq), np.asarray(Wk),
                            np.asarray(bk), np.asarray(Wv), np.asarray(bv),
                            np.asarray(pos_bias))
    res = bass_utils.run_bass_kernel_spmd(
        nc, in_maps, core_ids=list(range(8)), trace=TRACE,
    )
    LAST_RESULTS = res
    return _unshard(res.results)


# revision 58
# speedup vs baseline: 1.0121x; 1.0121x over previous
"""AFT-Local sparse attention kernel for Trainium2, SPMD over 8 NeuronCores.

Problem (B=4, L=1024, E=256, S=32):
    Q = q @ Wq.T + bq ; K = q @ Wk.T + bk ; V = q @ Wv.T + bv
    For each (b, i, e):  per-channel softmax over the 65-wide window
        logits[j] = Q[i,e] * (K[i+j-S, e] + pb[j, e])   for |j-S| < S (strict)
        logits[j] = 0                                    for j in {0, 64} (K masked)
        logits[j] = -inf                                 for out-of-range positions
        ctx = sum_j softmax(logits)[j] * V[i+j-S, e]
    out = sigmoid(Q)^2 * ctx

Sharding: 8 cores = (batch b in 0..3) x (sequence half h in 0..1).
The h=1 half is REVERSED on the host so that every core sees an identical
problem: a sequence edge at local position 0 and valid data through the
right halo.  This keeps the SPMD graph uniform (no per-core masking).

Device layout: channels on partitions (2 halves of 128), sequence on the
free axis.  Window shifts are free AP offsets.  Per window offset d:
  DVE/ACT:  t_d = K<<d + pb[d]             (tensor_scalar slabs, split
                                            between the two engines)
  DVE:  l_d = t_d * Q                      (one fused G-slab tensor_tensor)
  ACT:  E_d = exp(l_d)
  DVE:  EV_d = E_d * V<<d
  PE:   D += I.T @ E_d ; N += I.T @ EV_d   (identity matmuls accumulate in
        PSUM; D-matmuls first so ln(D) overlaps the N accumulation)
Final: out = sigmoid(Q)^2 * N / D.

The hot path runs in bf16 (measured end-to-end error ~1e-2 vs the 2e-2
gate).  K also exists as a 1-element-shifted copy so reads at odd window
offsets stay 4-byte aligned (keeps the DVE 2x packed mode).

Raw Bass (manual semaphores): this walrus build rejects Tile's generated
sync (multi-wait instructions), so engine programs and cumulative
wait_ge thresholds are written out explicitly.
"""

import contextlib

import ml_dtypes
import numpy as np

import concourse.bass as bass
import concourse.mybir as mybir
from concourse import bass_utils

# Re-enable walrus's LDWEIGHTS elision: the window-sum accumulation issues
# 250+ matmuls with the same identity lhsT, and the default
# --enable-ldw-opt=false reloads it every time.
if not getattr(bass_utils, "_ldw_opt_patched", False):
    bass_utils._ldw_opt_patched = True
    _orig_run_command = bass_utils.run_command

    def _run_command_ldw(argv, **kwargs):
        argv = ["--enable-ldw-opt=true" if a == "--enable-ldw-opt=false"
                else a for a in argv]
        return _orig_run_command(argv, **kwargs)

    bass_utils.run_command = _run_command_ldw

B, L, E, S = 4, 1024, 256, 32
O = 512          # output positions per core
HALO = 32        # halo on each side of the output range
NH = O + 2 * HALO  # 576: local K/V/q array length
P = 128
W = 2 * S + 1
F32 = mybir.dt.float32
BF16 = mybir.dt.bfloat16
NPBF = ml_dtypes.bfloat16

G = 7       # window offsets processed per block
NBLK = 18   # 9 blocks per channel half

TRACE = False
LAST_RESULTS = None
_DEBUG_TAP = None
_CACHE = {}

# ---- static semaphore bookkeeping ----
# sem_pe counts: 24 proj matmuls, then per eh: 3 init + 14 per block
PE_PROJ = 24


def n_act_slabs(b):
    """K+pb slabs done by the ACT engine for block b (rest on the DVE)."""
    if b < 2:
        return 0
    return 2 if b % 2 == 0 else 3


def blk_iters(b):
    return b // 9, -S + 1 + G * (b % 9)


def pe_after_block(b):
    return PE_PROJ + (3 if b < 9 else 6) + 14 * (b + 1)


def pe_d_close(b):
    """PE count at which block b's D-side matmuls have retired (D first)."""
    return pe_after_block(b) - G


PE_TOTAL = PE_PROJ + 6 + 14 * NBLK

# Projection copy-outs: groups 0-1 (eh0 k, FIRST so the K1 shift DMA and
# the window slabs can start as early as possible) on the DVE (s_pd);
# groups 2-3 (eh0 q) and 4-11 on the ScalarE (s_prj; v-chunk0 groups emit
# 2 ops).
ACT_CUM = {2: 1, 3: 2, 4: 4, 5: 5, 6: 6, 7: 7, 8: 8, 9: 9, 10: 11, 11: 12}


def _build_nc():
    nc = bass.Bass("TRN2")

    ACOLS = 2 * (NH + E)        # 1664: qT0 | Wq0 | qT1 | Wq1 per row
    BCOLS = 4 * E               # 1024: Wk0 | Wv0 | Wk1 | Wv1 per row
    FCOLS = W + 3               # 68:   pbT | biases
    blobA_d = nc.dram_tensor("blobA", [P, ACOLS], BF16, kind="ExternalInput")
    blobB_d = nc.dram_tensor("blobB", [P, BCOLS], BF16, kind="ExternalInput")
    # per-partition scalar operands must be f32 (ISA requirement)
    pbblob_d = nc.dram_tensor("pbblob", [P, 2 * W], F32, kind="ExternalInput")
    bblob_d = nc.dram_tensor("bblob", [P, 6], F32, kind="ExternalInput")
    out_dt = F32 if _DEBUG_TAP is not None else BF16
    out_d = nc.dram_tensor("out", [E, O], out_dt, kind="ExternalOutput")

    add = mybir.AluOpType.add
    mult = mybir.AluOpType.mult
    AF = mybir.ActivationFunctionType

    ctx = contextlib.ExitStack()
    with ctx:
        sb = lambda name, shape, dt=BF16: ctx.enter_context(
            nc.sbuf_tensor(name, shape, dt))[:, :]
        ps = lambda name, shape: ctx.enter_context(
            nc.psum_tensor(name, shape, F32))[:, :]
        sem = lambda name: ctx.enter_context(nc.semaphore(name))

        blobA = sb("blobA_s", [P, 2 * (NH + E)])
        blobB = sb("blobB_s", [P, 4 * E])
        pbs = sb("pbs", [P, 2 * W], F32)
        bss = sb("bss", [P, 6], F32)
        qT = [blobA[:, kh * (NH + E):kh * (NH + E) + NH] for kh in range(2)]
        wT = {("q", kh): blobA[:, kh * (NH + E) + NH:(kh + 1) * (NH + E)]
              for kh in range(2)}
        for kh in range(2):
            wT["k", kh] = blobB[:, 2 * kh * E:(2 * kh + 1) * E]
            wT["v", kh] = blobB[:, (2 * kh + 1) * E:(2 * kh + 2) * E]
        pb = [pbs[:, eh * W:(eh + 1) * W] for eh in range(2)]
        bs = [bss[:, eh * 3:(eh + 1) * 3] for eh in range(2)]
        QKV = {(t, eh): sb(f"{t}{eh}", [P, NH])
               for t in "qkv" for eh in range(2)}
        # 1-element-shifted copy for odd window offsets (alignment)
        K1 = [sb(f"k1_{eh}", [P, NH]) for eh in range(2)]
        ident = sb("ident", [P, P])
        ones2 = sb("ones2", [P, O])
        tbig = [sb(f"tbig{i}", [P, G * O]) for i in range(2)]
        lbig = [sb(f"lbig{i}", [P, G * O]) for i in range(4)]
        ebig = [sb(f"ebig{i}", [P, G * O]) for i in range(4)]
        vbig = [sb(f"vbig{i}", [P, G * O]) for i in range(4)]
        epT = [sb(f"epT{eh}", [P, O], F32) for eh in range(2)]
        epLD = [sb(f"epLD{eh}", [P, O], F32) for eh in range(2)]
        epLU = [sb(f"epLU{eh}", [P, O], F32) for eh in range(2)]
        epS = [sb(f"epS{eh}", [P, O], F32) for eh in range(2)]
        ob = [sb(f"ob{eh}", [P, O], out_dt) for eh in range(2)]
        tapb = sb("tapb", [P, O], F32)

        prj_ps = [ps(f"prj_ps{i}", [P, O]) for i in range(2)]
        D_ps = [ps(f"D_ps{eh}", [P, O]) for eh in range(2)]
        N_ps = [ps(f"N_ps{eh}", [P, O]) for eh in range(2)]

        s_pk = sem("s_pk")  # K1 shifted copies (sync-engine DMA)
        s_lA = sem("s_lA")  # blobA: qT + Wq (2 DMAs, 16 each)
        s_lC = sem("s_lC")  # blobB: Wk + Wv
        s_lB = sem("s_lB")  # bias + pos_bias
        s_gp = sem("s_gp")
        s_prj = sem("s_prj")
        s_pd = sem("s_pd")
        s_lg = sem("s_lg")
        s_ex = sem("s_ex")
        s_ev = sem("s_ev")
        s_pe = sem("s_pe")
        s_ea = sem("s_ea")
        s_ed = sem("s_ed")
        s_epi = sem("s_epi")
        s_od = sem("s_od")
        s_at = sem("s_at")

        def k_sh(eh, d):
            """K window-shifted AP, 4B-aligned: even offsets from K, odd
            from the 1-shifted copy."""
            o = HALO + d
            if o % 2 == 0:
                return QKV["k", eh][:, o:o + O]
            return K1[eh][:, o - 1:o - 1 + O]

        # projection groups: (eh, t, (n0, nn)); eh0 does k before q so the
        # window pipeline's K-dependent head starts earliest
        groups = [(eh, t, c) for eh, ts in ((0, "kqv"), (1, "qkv"))
                  for t in ts for c in ((0, 512), (512, NH - 512))]

        with nc.Block() as block:

            @block.sync
            def _(sync):
                # Consolidated loads: per-partition rows are contiguous
                # in DRAM so each DMA moves large packets. blobA is the
                # critical one (projections): split in two so both halves
                # transfer in parallel.
                HC = NH + E
                sync.dma_start(out=blobA[:, 0:HC], in_=blobA_d[:, 0:HC]
                               ).then_inc(s_lA, 16)
                sync.dma_start(out=blobA[:, HC:2 * HC],
                               in_=blobA_d[:, HC:2 * HC]).then_inc(s_lA, 16)
                sync.dma_start(out=blobB, in_=blobB_d[:, :]
                               ).then_inc(s_lC, 16)
                sync.dma_start(out=pbs, in_=pbblob_d[:, :]).then_inc(s_lB, 16)
                sync.dma_start(out=bss, in_=bblob_d[:, :]).then_inc(s_lB, 16)
                # K1 shifted copies as SBUF->SBUF DMAs: frees the DVE.
                sync.wait_ge(s_pd, 2)
                sync.dma_start(out=K1[0][:, 0:NH - 1],
                               in_=QKV["k", 0][:, 1:NH]).then_inc(s_pk, 16)
                sync.wait_ge(s_prj, 9)
                sync.dma_start(out=K1[1][:, 0:NH - 1],
                               in_=QKV["k", 1][:, 1:NH]).then_inc(s_pk, 16)
                if _DEBUG_TAP is None:
                    H = O // 2
                    sync.wait_ge(s_epi, 1)
                    sync.dma_start(out=out_d[0:P, :], in_=ob[0]
                                   ).then_inc(s_od, 16)
                    # eh1 leaves in two halves so the first DMA overlaps
                    # the second half's multiply.
                    sync.wait_ge(s_epi, 2)
                    sync.dma_start(out=out_d[P:2 * P, 0:H], in_=ob[1][:, 0:H]
                                   ).then_inc(s_od, 16)
                    sync.wait_ge(s_epi, 3)
                    sync.dma_start(out=out_d[P:2 * P, H:O], in_=ob[1][:, H:O]
                                   ).then_inc(s_od, 16)
                    sync.wait_ge(s_od, 48)
                else:
                    sync.wait_ge(s_epi, 3)
                    tap = {
                        "D0": lambda: tapb,
                        "N0": lambda: tapb,
                        "out0": lambda: ob[0],
                    }[_DEBUG_TAP]()
                    tw = tap.shape[1]
                    sync.dma_start(out=out_d[0:P, 0:tw], in_=tap
                                   ).then_inc(s_od, 16)
                    sync.wait_ge(s_od, 16)

            @block.gpsimd
            def _(gpsimd):
                gpsimd.memset(ident, 0.0)
                gpsimd.affine_select(
                    out=ident, in_=ident,
                    compare_op=mybir.AluOpType.not_equal,
                    fill=1.0, base=0, pattern=[[-1, P]], channel_multiplier=1,
                ).then_inc(s_gp, 1)
                # D-init vector: 1 in the left-edge region (r=-S invalid),
                # 2 elsewhere (both window-edge exp(0) terms).
                gpsimd.memset(ones2, 2.0)
                gpsimd.memset(ones2[:, 0:HALO], 1.0).then_inc(s_gp, 1)

            @block.tensor
            def _(tensor):
                # projections: 4-deep ping-pong (the D/N eh0 banks are free
                # until the window phase starts, which waits s_prj>=10)
                tensor.wait_ge(s_lA, 32)
                kv_waited = False
                pbanks = [prj_ps[0], prj_ps[1], D_ps[0], N_ps[0]]
                for g, (eh, t, (n0, nn)) in enumerate(groups):
                    bank = pbanks[g % 4]
                    if t != "q" and not kv_waited:
                        kv_waited = True
                        tensor.wait_ge(s_lC, 16)
                    if g >= 4:
                        dep = g - 4
                        if dep in (0, 1):
                            tensor.wait_ge(s_pd, dep + 1)
                        else:
                            tensor.wait_ge(s_prj, ACT_CUM[dep])
                    for kh in range(2):
                        tensor.matmul(
                            bank[:, :nn],
                            wT[t, kh][:, eh * P:(eh + 1) * P],
                            qT[kh][:, n0:n0 + nn],
                            start=(kh == 0), stop=(kh == 1),
                        ).then_inc(s_pe, 1)
                # window accumulation, one block of G offsets at a time.
                # Per block: D-side matmuls first (gated on exp only), so
                # the D accumulation closes G matmuls before N and the
                # ln(D) epilogue overlaps the N accumulation.
                for b in range(NBLK):
                    eh, d0 = blk_iters(b)
                    if b % 9 == 0:
                        V = QKV["v", eh]
                        if b == 0:
                            tensor.wait_ge(s_gp, 2)
                        # s_prj>=12 also guarantees the proj bank aliasing
                        # of D_ps[0]/N_ps[0] is fully drained.
                        tensor.wait_ge(s_prj, 12)
                        # r=+S edge term (always valid) zeroes the banks,
                        # then the r=-S edge term and the 1/2 D-init.
                        tensor.matmul(N_ps[eh], ident,
                                      V[:, 2 * HALO:2 * HALO + O],
                                      start=True, stop=False).then_inc(s_pe, 1)
                        tensor.matmul(N_ps[eh][:, HALO:], ident,
                                      V[:, HALO:O],
                                      start=False, stop=False).then_inc(s_pe, 1)
                        tensor.matmul(D_ps[eh], ident, ones2,
                                      start=True, stop=False).then_inc(s_pe, 1)
                    tensor.wait_ge(s_ex, b + 1)
                    for g in range(G):
                        d = d0 + g
                        vs = max(0, -d)
                        tensor.matmul(D_ps[eh][:, vs:], ident,
                                      ebig[b % 4][:, g * O + vs:(g + 1) * O],
                                      start=False, stop=(d == S - 1)
                                      ).then_inc(s_pe, 1)
                    tensor.wait_ge(s_ev, b + 1)
                    for g in range(G):
                        if b == NBLK - 1 and g == 4:
                            tensor.wait_ge(s_ev, b + 2)  # split last EV
                        d = d0 + g
                        vs = max(0, -d)
                        tensor.matmul(N_ps[eh][:, vs:], ident,
                                      vbig[b % 4][:, g * O + vs:(g + 1) * O],
                                      start=False, stop=(d == S - 1)
                                      ).then_inc(s_pe, 1)

            @block.vector
            def _(vector):
                def emit_ev_block(bb, g0=0, gn=G):
                    ehb, d0b = blk_iters(bb)
                    if g0 == 0:
                        vector.wait_ge(s_ex, bb + 1)
                        if bb >= 3 and bb % 2 == 1:
                            # vbig slot free; 4-buffered, so one wait at odd
                            # bb covers this block (needs bb-4) and the next
                            # (needs bb-3).
                            vector.wait_ge(s_pe, pe_after_block(bb - 3))
                    # single op: misaligned packed reads measured full-speed
                    ng = gn - g0
                    vsrc = QKV["v", ehb]
                    in1 = bass.AP(
                        tensor=vsrc.tensor,
                        offset=vsrc.offset + HALO + d0b + g0,
                        ap=[vsrc.ap[0], [1, ng], [1, O]])
                    e3 = bass.AP(
                        tensor=ebig[bb % 4].tensor,
                        offset=ebig[bb % 4].offset + g0 * O,
                        ap=[ebig[bb % 4].ap[0], [O, ng], [1, O]])
                    v3 = bass.AP(
                        tensor=vbig[bb % 4].tensor,
                        offset=vbig[bb % 4].offset + g0 * O,
                        ap=[vbig[bb % 4].ap[0], [O, ng], [1, O]])
                    vector.tensor_tensor(out=v3, in0=e3, in1=in1, op=mult
                                         ).then_inc(s_ev, 1)

                vector.wait_ge(s_lB, 32)  # bias scalars present
                pbanks = [prj_ps[0], prj_ps[1], D_ps[0], N_ps[0]]
                for g in (0, 1):
                    ehg, tg, (n0, nn) = groups[g]
                    ti = "qkv".index(tg)
                    vector.wait_ge(s_pe, 2 * (g + 1))
                    vector.tensor_scalar_add(
                        QKV[tg, ehg][:, n0:n0 + nn], pbanks[g % 4][:, :nn],
                        bs[ehg][:, ti:ti + 1]).then_inc(s_pd, 1)
                for b in range(NBLK):
                    eh, d0 = blk_iters(b)
                    Q = QKV["q", eh]
                    gs = list(range(G - n_act_slabs(b)))
                    if b % 9 == 0:
                        if b == 9:
                            vector.wait_ge(s_prj, 9)  # K+Q eh1 from the ACT
                        # do the direct-K slabs while the K1 DMA lands
                        gs = ([g for g in gs if (HALO + d0 + g) % 2 == 0]
                              + [None]
                              + [g for g in gs if (HALO + d0 + g) % 2 == 1])
                    for g in gs:
                        if g is None:
                            vector.wait_ge(s_pk, 16 * (eh + 1))  # K1 DMA
                            continue
                        d = d0 + g
                        vector.tensor_scalar_add(
                            tbig[b % 2][:, g * O:(g + 1) * O], k_sh(eh, d),
                            pb[eh][:, d + S:d + S + 1])
                    if b == 0:
                        vector.wait_ge(s_prj, 2)  # Q eh0 from the ACT
                    if b >= 2:
                        vector.wait_ge(s_at, b - 1)  # ACT-side K+pb slabs
                    if b >= 4 and b % 2 == 0:
                        # lbig slot free; one wait at even b covers b
                        # (needs s_ex >= b-3) and b+1 (needs b-2).
                        vector.wait_ge(s_ex, b - 2)
                    qb = bass.AP(
                        tensor=Q.tensor, offset=Q.offset + HALO,
                        ap=[Q.ap[0], [0, G], [1, O]])
                    tb3 = bass.AP(
                        tensor=tbig[b % 2].tensor, offset=tbig[b % 2].offset,
                        ap=[tbig[b % 2].ap[0], [O, G], [1, O]])
                    lb3 = bass.AP(
                        tensor=lbig[b % 4].tensor, offset=lbig[b % 4].offset,
                        ap=[lbig[b % 4].ap[0], [O, G], [1, O]])
                    vector.tensor_tensor(out=lb3, in0=tb3, in1=qb, op=mult
                                         ).then_inc(s_lg, 1)
                    if b == 2:
                        vector.wait_ge(s_prj, 5)   # V eh0 for the EV mult
                    if b == 11:
                        vector.wait_ge(s_prj, 12)  # V eh1 for the EV mult
                    if b >= 2:
                        emit_ev_block(b - 2)
                    if b == 2:
                        for eh in range(2):
                            vector.wait_ge(s_ea, eh + 1)
                            vector.tensor_scalar_add(epS[eh], epT[eh], 1.0
                                                     ).then_inc(s_ed, 1)
                    if b == 13:
                        vector.wait_ge(s_ea, 5)
                        vector.scalar_tensor_tensor(
                            out=epT[0], in0=epLU[0], scalar=2.0,
                            in1=epLD[0], op0=mult, op1=add,
                        ).then_inc(s_ed, 1)
                    if b == 15:
                        vector.wait_ge(s_ea, 6)
                        vector.wait_ge(s_pe, pe_after_block(8))  # N0 done
                        vector.tensor_mul(ob[0], N_ps[0], epS[0]
                                          ).then_inc(s_epi, 1)
                emit_ev_block(NBLK - 2)
                # last block's EV in two halves so the PE's final N matmuls
                # start one half earlier
                emit_ev_block(NBLK - 1, 0, 4)
                emit_ev_block(NBLK - 1, 4, G)

                # tail: out = N * exp(-(ln D + 2 ln(1+exp(-Q))))
                if _DEBUG_TAP in ("D0", "N0"):
                    vector.wait_ge(s_pe, PE_TOTAL)
                    vector.tensor_copy(
                        tapb, D_ps[0] if _DEBUG_TAP == "D0" else N_ps[0])
                H = O // 2
                vector.wait_ge(s_ea, 7)
                vector.scalar_tensor_tensor(
                    out=epT[1], in0=epLU[1], scalar=2.0,
                    in1=epLD[1], op0=mult, op1=add,
                ).then_inc(s_ed, 1)
                vector.wait_ge(s_ea, 8)
                vector.wait_ge(s_pe, PE_TOTAL)  # N1 done
                vector.tensor_mul(ob[1][:, 0:H], N_ps[1][:, 0:H],
                                  epS[1][:, 0:H]).then_inc(s_epi, 1)
                vector.wait_ge(s_ea, 9)
                vector.tensor_mul(ob[1][:, H:O], N_ps[1][:, H:O],
                                  epS[1][:, H:O]).then_inc(s_epi, 1)

            @block.scalar
            def _(scalar):
                # projections: add bias, move PSUM -> SBUF (groups 4-11)
                scalar.wait_ge(s_lB, 32)  # bias + pos_bias present
                pbanks = [prj_ps[0], prj_ps[1], D_ps[0], N_ps[0]]
                for g in (2, 3, *range(4, 12)):
                    eh, t, (n0, nn) = groups[g]
                    ti = "qkv".index(t)
                    bank = pbanks[g % 4]
                    scalar.wait_ge(s_pe, 2 * (g + 1))
                    T_sb = QKV[t, eh]
                    if t == "v" and n0 == 0:
                        scalar.activation(T_sb[:, 0:HALO], bank[:, 0:HALO],
                                          AF.Copy).then_inc(s_prj, 1)
                        scalar.activation(
                            T_sb[:, HALO:nn], bank[:, HALO:nn], AF.Identity,
                            bias=bs[eh][:, ti:ti + 1], scale=1.0,
                        ).then_inc(s_prj, 1)
                    else:
                        scalar.activation(
                            T_sb[:, n0:n0 + nn], bank[:, :nn], AF.Identity,
                            bias=bs[eh][:, ti:ti + 1], scale=1.0,
                        ).then_inc(s_prj, 1)
                    if g == 9:
                        # exp(0) as soon as the first logits land; the
                        # remaining copies and sigma-side ops are off the
                        # critical path.
                        scalar.wait_ge(s_lg, 1)
                        scalar.activation(ebig[0], lbig[0], AF.Exp
                                          ).then_inc(s_ex, 1)
                for eh in range(2):
                    scalar.activation(epT[eh], QKV["q", eh][:, HALO:HALO + O],
                                      AF.Exp, scale=-1.0).then_inc(s_ea, 1)
                for b in range(NBLK):
                    ehb, d0b = blk_iters(b)
                    if b >= 2:
                        if b == 2:
                            scalar.wait_ge(s_pk, 16)  # eh0 K1 via DMA
                        if b == 9:
                            scalar.wait_ge(s_pk, 32)  # eh1 K1 via DMA
                        scalar.wait_ge(s_lg, b - 1)  # tbig slot free
                        for g in range(G - n_act_slabs(b), G):
                            d = d0b + g
                            ai = scalar.activation(
                                tbig[b % 2][:, g * O:(g + 1) * O],
                                k_sh(ehb, d), AF.Identity,
                                bias=pb[ehb][:, d + S:d + S + 1], scale=1.0)
                        ai.then_inc(s_at, 1)
                    if b >= 2:  # exp(0) was hoisted above the sigma ops
                        bb = b - 1
                        scalar.wait_ge(s_lg, bb + 1)
                        if bb >= 4 and bb % 2 == 0:
                            # ebig slot free (covers PE reads and the DVE EV
                            # of bb-4, transitively); one wait at even bb
                            # covers bb (needs bb-4) and bb+1 (needs bb-3).
                            scalar.wait_ge(s_pe, pe_after_block(bb - 3))
                        scalar.activation(ebig[bb % 4], lbig[bb % 4], AF.Exp
                                          ).then_inc(s_ex, 1)
                        if bb == 4:
                            # LU = ln(1+exp(-Q)), both eh (u ready early)
                            for eh in range(2):
                                scalar.wait_ge(s_ed, eh + 1)
                                scalar.activation(epLU[eh], epS[eh], AF.Ln
                                                  ).then_inc(s_ea, 1)
                        if bb == 10:
                            scalar.wait_ge(s_pe, pe_d_close(8))
                            scalar.activation(epLD[0], D_ps[0], AF.Ln
                                              ).then_inc(s_ea, 1)
                        if bb == 12:
                            scalar.wait_ge(s_ed, 3)
                            scalar.activation(epS[0], epT[0], AF.Exp,
                                              scale=-1.0).then_inc(s_ea, 1)
                bb = NBLK - 1
                scalar.wait_ge(s_lg, bb + 1)
                # ebig slot covered by the parity wait at bb-1
                scalar.activation(ebig[bb % 4], lbig[bb % 4], AF.Exp
                                  ).then_inc(s_ex, 1)
                # epilogue, same exp/ln table set (no set switch).
                # sigma-side (T = exp(-Q), LU = ln(1+T)) runs early; the
                # D-dependent part starts as soon as block 17's D-side
                # matmuls retire, overlapping the N accumulation.
                scalar.wait_ge(s_pe, pe_d_close(NBLK - 1))
                scalar.activation(epLD[1], D_ps[1], AF.Ln).then_inc(s_ea, 1)
                Hh = O // 2
                scalar.wait_ge(s_ed, 4)
                scalar.activation(epS[1][:, 0:Hh], epT[1][:, 0:Hh], AF.Exp,
                                  scale=-1.0).then_inc(s_ea, 1)
                scalar.activation(epS[1][:, Hh:O], epT[1][:, Hh:O], AF.Exp,
                                  scale=-1.0).then_inc(s_ea, 1)

    return nc


def _shard_inputs(q, Wq, bq, Wk, bk, Wv, bv, pos_bias):
    """Build per-core input maps. Core c = 2*b + h."""
    wqT = Wq.T.astype(NPBF)
    wkT = Wk.T.astype(NPBF)
    wvT = Wv.T.astype(NPBF)
    bias = np.stack([bq, bk, bv], axis=1).astype(np.float32)   # [E, 3]
    bblob = np.ascontiguousarray(
        np.concatenate([bias[0:P], bias[P:E]], axis=1))        # [P, 6]
    pbT_f = pos_bias.T.astype(np.float32)                      # [E, W]
    pbT_r = pos_bias[::-1].T.astype(np.float32)                # reversed

    blobB = np.ascontiguousarray(np.concatenate(
        [wkT[0:P], wvT[0:P], wkT[P:E], wvT[P:E]], axis=1))     # [P, 4E]
    in_maps = []
    for c in range(8):
        b, h = divmod(c, 2)
        qh = np.zeros((NH, E), np.float32)
        if h == 0:
            qh[HALO:] = q[b, 0:O + HALO]          # positions -32..543, pad<0
        else:
            qh[HALO:] = q[b, L - (O + HALO):][::-1]  # reversed right half
        qT = qh.T.astype(NPBF)                                 # [E, NH]
        pbT = pbT_f if h == 0 else pbT_r
        in_maps.append({
            "blobA": np.ascontiguousarray(np.concatenate(
                [qT[0:P], wqT[0:P], qT[P:E], wqT[P:E]], axis=1)),
            "blobB": blobB,
            "pbblob": np.ascontiguousarray(np.concatenate(
                [pbT[0:P], pbT[P:E]], axis=1)),                # [P, 2W]
            "bblob": bblob,
        })
    return in_maps


def _unshard(results):
    out = np.empty((B, L, E), np.float32)
    for c in range(8):
        b, h = divmod(c, 2)
        o_core = np.asarray(results[c]["out"]).astype(np.float32).T  # [O, E]
        if h == 0:
            out[b, 0:O] = o_core
        else:
            out[b, L - O:] = o_core[::-1]
    return out


def kernel(q, Wq, bq, Wk, bk, Wv, bv, pos_bias):
    global LAST_RESULTS
    q = np.asarray(q, np.float32)
    if "nc" not in _CACHE:
        _CACHE["nc"] = _build_nc()
    nc = _CACHE["nc"]
    in_maps = _shard_inputs(q, np.asarray(Wq), np.asarray(bq), np.asarray(Wk),
                            np.asarray(bk), np.asarray(Wv), np.asarray(bv),
                            np.asarray(pos_bias))
    res = bass_utils.run_bass_kernel_spmd(
        nc, in_maps, core_ids=list(range(8)), trace=TRACE,
    )
    LAST_RESULTS = res
    return _unshard(res.results)
